# revision 28
# baseline (speedup 1.0000x reference)
"""Trainium2 Bass kernel for nn_Attention_88888643158200.

Single-head NCH attention, B=8, C=256, N=2048:
    logits[b,i,j] = sum_c k[b,c,i] q[b,c,j] / sqrt(C)
    weights = softmax(logits, axis=i)            # normalize over key index
    out[b,c,j]   = sum_i v[b,c,i] weights[b,i,j]
Returns (out [B,C,N], weights [B,N,N]).

Sharding: data-parallel over B across 8 NeuronCores (1 batch per core),
no cross-core communication.

Per-core design (all matmuls float32r = fp32 data read as FP22 by the PE,
1 cycle/row; fp32 accumulation in PSUM):
  phase 1   logits tile L[i_p=128, j_f=512] = k[c,i_blk].T @ q[c,j_chunk]
            (2 accumulating matmuls over the c=256 contraction)
  exp       E = exp(SCALE * L) on ScalarE (PSUM -> SBUF), no max-subtract
            (logits ~ N(0,1), max ~ 6, exp is fp32-safe)
  denom     D[1,512] += ones[128,1].T @ E  (PE, accumulated over 16 i-blocks)
  recip     r = 1/D via reciprocal_approx_fast (DVE)
  bcast     R[128,512] = ones[1,128].T @ r[1,512] (PE) -> SBUF via ScalarE
  weights   W = E * R (VectorE/GpSimd split), DMA to HBM
  out       O[c,j] = vT[i,c].T @ E[i,j] accumulated over i, then * R (DVE)
vT comes from 32 PE transposes of v at kernel start.
"""

import numpy as np
from contextlib import ExitStack

B, C, N = 8, 256, 2048
P = 128
JC = 512                   # j-chunk width (one PSUM bank of fp32)
NCHUNK = N // JC           # 4
NIB = N // P               # 16 i-blocks
SCALE = float(1.0 / np.sqrt(np.float32(C)))

_CACHE = {}


def _build_program(repeats: int = 1):
    import concourse.bass as bass  # noqa: F401
    import concourse.tile as tile
    from concourse import bacc, mybir
    from concourse.masks import make_identity

    f32 = mybir.dt.float32
    f32r = mybir.dt.float32r
    EXP = mybir.ActivationFunctionType.Exp

    nc = bacc.Bacc(
        "TRN2",
        target_bir_lowering=False,
        debug=False,
        num_devices=8,
    )

    q_d = nc.dram_tensor("q", [C, N], f32, kind="ExternalInput").ap()
    k_d = nc.dram_tensor("k", [C, N], f32, kind="ExternalInput").ap()
    v_d = nc.dram_tensor("v", [C, N], f32, kind="ExternalInput").ap()
    out_d = nc.dram_tensor("out", [C, N], f32, kind="ExternalOutput").ap()
    w_d = nc.dram_tensor("weights", [N, N], f32, kind="ExternalOutput").ap()

    # DRAM views with the 128-partition dim explicit
    q_v = q_d.rearrange("(h p) n -> p h n", p=P)      # [128, 2, 2048]
    k_v = k_d.rearrange("(h p) n -> p h n", p=P)
    v_v = v_d.rearrange("(h p) n -> p h n", p=P)
    out_v = out_d.rearrange("(h p) n -> p h n", p=P)
    w_v = w_d.rearrange("(ib p) j -> p ib j", p=P)    # [128, 16, 2048]

    with tile.TileContext(nc) as tc, ExitStack() as ctx:
        consts = ctx.enter_context(tc.tile_pool(name="consts", bufs=1))
        inp = ctx.enter_context(tc.tile_pool(name="inp", bufs=1))
        epool = ctx.enter_context(tc.tile_pool(name="epool", bufs=2))
        wpool = ctx.enter_context(tc.tile_pool(name="wpool", bufs=4))
        rpool = ctx.enter_context(tc.tile_pool(name="rpool", bufs=2))
        opool = ctx.enter_context(tc.tile_pool(name="opool", bufs=3))
        psumL = ctx.enter_context(tc.tile_pool(name="psumL", bufs=5, space="PSUM"))
        psumD = ctx.enter_context(tc.tile_pool(name="psumD", bufs=1, space="PSUM"))
        psumR = ctx.enter_context(tc.tile_pool(name="psumR", bufs=1, space="PSUM"))
        psumO = ctx.enter_context(tc.tile_pool(name="psumO", bufs=1, space="PSUM"))

        ident = consts.tile([P, P], f32)
        make_identity(nc, ident)
        ones_col_f = consts.tile([P, 1], f32)
        nc.vector.memset(ones_col_f, 1.0)
        ones_col = consts.tile([P, 1], f32r)
        nc.vector.tensor_copy(ones_col, ones_col_f)
        ones_row_f = consts.tile([1, P], f32)
        nc.vector.memset(ones_row_f, 1.0)
        ones_row = consts.tile([1, P], f32r)
        nc.vector.tensor_copy(ones_row, ones_row_f)

        for _rep in range(repeats):
            Q = inp.tile([P, 2, N], f32r, tag="Q")
            K = inp.tile([P, 2, N], f32r, tag="K")
            V = inp.tile([P, 2, N], f32, tag="V")
            sv0 = slice(0, 128)
            nc.sync.dma_start(out=V[:, :, sv0], in_=v_v[:, :, sv0])
            kb = [0, 128, 256, 512, 1024, 1536, 2048]
            qb = [0, 384, 896, 1408, 1792, 2048]
            kq = [("K", kb[s], kb[s + 1]) for s in range(len(kb) - 1)]
            for s in range(len(qb) - 1):
                kq.insert(2 * s + 1, ("Q", qb[s], qb[s + 1]))
            for nm, lo, hi in kq:
                t_, v_ = (K, k_v) if nm == "K" else (Q, q_v)
                nc.sync.dma_start(out=t_[:, :, lo:hi], in_=v_[:, :, lo:hi].bitcast(f32r))
            vb = [128, 512, 1024, 2048]
            for s in range(len(vb) - 1):
                ss = slice(vb[s], vb[s + 1])
                nc.sync.dma_start(out=V[:, :, ss], in_=v_v[:, :, ss])

            # vT[p, ib, c] = v[c, ib*128 + p]
            vT = inp.tile([P, NIB, C], f32r, tag="vT")
            for t in range(NIB // 2):
                pT = psumO.tile([P, 4, P], f32, tag="psO", name="pT")
                for u in range(2):
                    ib = 2 * t + u
                    for cb in range(2):
                        nc.tensor.transpose(
                            pT[:, 2 * u + cb, :],
                            V[:, cb, ib * P:(ib + 1) * P],
                            ident,
                        )
                nc.vector.tensor_copy(
                    vT[:, 2 * t:2 * t + 2, :].rearrange("p a c -> p (a c)"),
                    pT.rearrange("p a c -> p (a c)"),
                )


            CW_LIST = [384, 512, 512, 384, 256]
            joff = 0
            for jc, CW in enumerate(CW_LIST):
                js = slice(joff, joff + CW)
                joff += CW

                E_t = epool.tile([P, NIB, JC], f32r, tag="E", name="E_t")
                E = E_t[:, :, :CW]
                pD_t = psumD.tile([1, JC], f32, name="pD_t")
                pD = pD_t[:, :CW]

                for ib in range(NIB):
                    pL_t = psumL.tile([P, JC], f32, name="pL_t")
                    pL = pL_t[:, :CW]
                    for cb in range(2):
                        nc.tensor.matmul(
                            pL,
                            K[:, cb, ib * P:(ib + 1) * P],
                            Q[:, cb, js],
                            start=(cb == 0),
                            stop=(cb == 1),
                        )
                    nc.scalar.activation(
                        E[:, ib, :],
                        pL,
                        EXP,
                        scale=SCALE,
                    )
                    nc.tensor.matmul(
                        pD,
                        ones_col,
                        E[:, ib, :],
                        start=(ib == 0),
                        stop=(ib == NIB - 1),
                    )

                r_row_t = rpool.tile([1, JC], f32, tag="r_row", name="r_row_t")
                r_row = r_row_t[:, :CW]
                nc.vector.reciprocal_approx_fast(out=r_row, in_=pD)
                r_row_r_t = rpool.tile([1, JC], f32r, tag="r_row_r", name="r_row_r_t")
                r_row_r = r_row_r_t[:, :CW]
                nc.vector.tensor_copy(r_row_r, r_row)
                pR_t = psumR.tile([P, JC], f32, name="pR_t")
                pR = pR_t[:, :CW]
                nc.tensor.matmul(pR, ones_row, r_row_r, start=True, stop=True)
                R_t = rpool.tile([P, JC], f32, tag="R", name="R_t")
                R = R_t[:, :CW]
                nc.scalar.copy(R, pR)

                # normalized weights out: W = E * R  (small tiles first so
                # the DMA stream starts early, then bigger ones to keep the
                # HWDGE dispatch count low; GpSimd takes one 4-block tile)
                wg = [(0, 2, "v"), (2, 2, "v"), (4, 4, "g"), (8, 4, "v"), (12, 4, "v")]
                for (ib0, g, e_) in wg:
                    W_t = wpool.tile([P, 4, JC], f32, tag="W", name="W_t")
                    W = W_t[:, :g, :CW]
                    eng = nc.gpsimd if e_ == "g" else nc.vector
                    eng.tensor_mul(
                        W,
                        E[:, ib0:ib0 + g, :],
                        R[:, None, :].broadcast_to([P, g, CW]),
                    )
                    nc.sync.dma_start(out=w_v[:, ib0:ib0 + g, js], in_=W)

                # out = (v @ E) * R
                Ot_t = opool.tile([P, 2, JC], f32, tag="Ot", name="Ot_t")
                Ot = Ot_t[:, :, :CW]
                for cb in range(2):
                    pO_t = psumO.tile([P, JC], f32, tag="psO", name="pO_t")
                    pO = pO_t[:, :CW]
                    for ib in range(NIB):
                        nc.tensor.matmul(
                            pO,
                            vT[:, ib, cb * P:(cb + 1) * P],
                            E[:, ib, :],
                            start=(ib == 0),
                            stop=(ib == NIB - 1),
                        )
                    nc.vector.tensor_mul(Ot[:, cb, :], pO, R)
                nc.sync.dma_start(out=out_v[:, :, js], in_=Ot)

    nc.compile()
    return nc


def get_program(repeats: int = 1):
    key = ("nc", repeats)
    if key not in _CACHE:
        _CACHE[key] = _build_program(repeats)
    return _CACHE[key]


def kernel(q: np.ndarray, k: np.ndarray, v: np.ndarray):
    from concourse.bass_utils import run_bass_kernel_spmd

    nc = get_program()
    in_maps = [
        {
            "q": np.ascontiguousarray(q[b], dtype=np.float32),
            "k": np.ascontiguousarray(k[b], dtype=np.float32),
            "v": np.ascontiguousarray(v[b], dtype=np.float32),
        }
        for b in range(B)
    ]
    res = run_bass_kernel_spmd(nc, in_maps, core_ids=list(range(B)))
    out = np.stack([res.results[b]["out"] for b in range(B)])
    weights = np.stack([res.results[b]["weights"] for b in range(B)])
    return out, weights
